# revision 26
# baseline (speedup 1.0000x reference)
"""MultiHeadAttention Trainium2 Bass kernel (B=8, S=1024, D=1024, H=16).

Sharding: data-parallel over batch — core b computes batch element b.

Per-core algorithm (all matmul inputs bf16, PSUM accumulation fp32):
  * Host prep: upload X_q^T, X_k^T, X_v^T (transposed activations), W_q/8,
    W_k, W_v — all bf16 — plus an additive key mask [128, 8] fp32 and a
    bf16 identity matrix (warm-up junk matmuls only).
  * Projections on PE: Q^T = (W_q/8)^T-stationary x X_q^T-moving -> [D, S];
    K^T likewise; V = X_v^T-stationary x W_v-moving -> [S, D] stored with a
    ones-column appended per head (V_aug[:, h*65+64] = 1).  V chains are
    split into two half-contractions (k 0-3 / 4-7) so PSUM banks release
    quickly while xv/wv still stream in; halves are summed on DVE.
  * Scores: per head-pair, kpos-chunk c, q-chunk qc: scores^T[kpos, q] =
    K_h-stationary x Q_h^T-moving, two heads packed in the PE array via
    tile_position row groups (K=64 each). PSUM fp32 [128, 1024].
  * Softmax: ONE ScalarE exp per chunk, additive -1e9 mask fused via the
    per-partition bias port; no max-subtraction (scores ~ N(0,1)); output
    bf16 P^T directly to SBUF.
  * attnV, pt-STATIONARY: po[q, dh] += (P^T block)^T-stationary x
    [V_h | ones]-moving, accumulated over c in PSUM ([128, 4, 65] per
    head). Column 64 = sum of exp (softmax denominator). Output lands
    directly in [q, d] orientation — no PE transposes needed at all.
  * Out-phase: reciprocal of the denominator column, per-partition
    tensor_scalar multiply into OP [128, 4, 128] fp32, DMA straight to
    the final [S, D] layout.
"""
import numpy as np
import ml_dtypes

import concourse.bass as bass
import concourse.mybir as mybir
import concourse.tile as tile
from concourse.bass_utils import run_bass_kernel_spmd

F32 = mybir.dt.float32
BF16 = mybir.dt.bfloat16
AF = mybir.ActivationFunctionType

B, S, D, H = 8, 1024, 1024, 16
DH = D // H          # 64
KT = 8               # contraction chunks of 128
NEG = -1.0e9
N_CORES = 8

_cache = {}


def _split_excess_waits(nc, limit: int = 1):
    """Walrus TPB instruction structs encode exactly ONE wait; hoist excess
    waits emitted by Tile into standalone InstEventSemaphore instructions."""
    ctr = 0
    for f in nc.m.functions:
        for bb in f.blocks:
            new = []
            changed = False
            for inst in bb.instructions:
                si = inst.sync_info
                waits = list(si.on_wait) if si is not None and si.on_wait else []
                if len(waits) > limit:
                    excess, keep = waits[:-limit], waits[-limit:]
                    for w in excess:
                        ctr += 1
                        new.append(mybir.InstEventSemaphore(
                            name=f"wsplit-{ctr}",
                            engine=inst.engine,
                            ins=[], outs=[],
                            sync_info=mybir.SyncInfo(on_wait=[w], on_update=[]),
                        ))
                    inst.sync_info = mybir.SyncInfo(
                        on_wait=keep,
                        on_update=list(si.on_update) if si.on_update else [],
                    )
                    changed = True
                new.append(inst)
            if changed:
                bb.instructions = new
    return ctr


def _build_program():
    nc = bass.Bass()
    xq = nc.declare_dram_parameter("xq", [D, S], BF16, isOutput=False)   # X_q^T
    xk = nc.declare_dram_parameter("xk", [D, S], BF16, isOutput=False)   # X_k^T
    xv = nc.declare_dram_parameter("xv", [D, S], BF16, isOutput=False)   # X_v^T
    wq = nc.declare_dram_parameter("wq", [D, D], BF16, isOutput=False)   # W_q/8
    wk = nc.declare_dram_parameter("wk", [D, D], BF16, isOutput=False)
    wv = nc.declare_dram_parameter("wv", [D, D], BF16, isOutput=False)
    msk = nc.declare_dram_parameter("msk", [128, KT], F32, isOutput=False)
    idn = nc.declare_dram_parameter("idn", [128, 128], BF16, isOutput=False)
    out = nc.declare_dram_parameter("out", [S, D], F32, isOutput=True)

    with tile.TileContext(nc) as tc:
        with (
            tc.tile_pool(name="persist", bufs=1) as pers,
            tc.tile_pool(name="xw", bufs=24) as xw,
            tc.tile_pool(name="pt", bufs=20) as ptp,
            tc.tile_pool(name="outp", bufs=3) as outp,
            tc.tile_pool(name="rr", bufs=8) as rrp,
            tc.tile_pool(name="pp", bufs=2, space="PSUM") as pp,
            tc.tile_pool(name="psc", bufs=2, space="PSUM") as psc,
            tc.tile_pool(name="pod", bufs=2, space="PSUM") as pod,
        ):
            # ---------- constants ----------
            mask_sb = pers.tile([128, KT], F32)
            nc.sync.dma_start(out=mask_sb, in_=msk[:, :])
            id_sb = pers.tile([128, 128], BF16)
            nc.sync.dma_start(out=id_sb, in_=idn[:, :])
            warm = pers.tile([128, 1], F32)
            nc.scalar.copy(warm, mask_sb[:, 0:1])            # warm ACT clock
            # HAM warm-up: junk matmuls on the identity tile while input DMAs
            # stream, so the first projection matmuls run at 2.4 GHz
            for _ in range(4):
                jw = pp.tile([32, 128], F32, tag="pp")
                for j in range(9):
                    nc.tensor.matmul(
                        jw[:, :], lhsT=id_sb[0:32, 0:32],
                        rhs=id_sb[0:32, 0:128],
                        start=(j == 0), stop=(j == 8))

            # ---------- persistent activations ----------
            QT = pers.tile([128, KT, S], BF16)     # Q^T tiles: rows 128r+p
            KTt = pers.tile([128, KT, S], BF16)    # K^T
            VA = pers.tile([128, KT, H * 65], BF16)  # V with ones columns

            def load_quarter(param, qtr):
                t = xw.tile([128, 2, S], BF16, tag="xw",
                            name=f"ld_{param.name}_{qtr}")
                nc.scalar.dma_start(
                    out=t, in_=param[:, :].rearrange(
                        "(a p) s -> p a s", p=128)[:, 2 * qtr:2 * qtr + 2, :])
                return t

            # Q/K activations+weights stream first (the scores->exp pipeline
            # is the pacer); V last (attnV tolerates lag via pt buffering).
            # The DMA engines round-robin packets across ALL enqueued
            # descriptors, so ungated pieces would all complete only at the
            # very end of the 12MB stream.  Gate: issue from the scalar
            # engine (idle until the first exp), 3 groups of 1MB in flight —
            # a tiny scalar copy READS group i-3 before group i is enqueued,
            # forcing sequential completion at full bandwidth.
            junk_g = pers.tile([128, 1], BF16)
            junk_s = pers.tile([1, 2], BF16)
            pieces = {}
            glast = []
            # Q/K groups: issued+gated on the scalar engine (idle until the
            # first exp, and all its gates clear before scores need K).
            for gi, (pmx, pmw) in enumerate([(xq, wq)] * 4 + [(xk, wk)] * 4):
                qtr = gi % 4
                if gi >= 3:
                    nc.scalar.copy(junk_g, glast[gi - 3][:, 0, 0:1])
                pieces[(pmx.name, qtr)] = load_quarter(pmx, qtr)
                pieces[(pmw.name, qtr)] = load_quarter(pmw, qtr)
                glast.append(pieces[(pmw.name, qtr)])
            # V groups: issued on gpsimd (idle until the first output DMA)
            # with gpsimd COMPUTE gates — a tensor_copy read waits for the
            # gated group's DMA DATA (a junk-DMA gate only waits for the
            # descriptor issue, which is useless).  First gate holds V until
            # the K stream has landed; then 2 groups in flight.
            def load_quarter_gp(param, qtr):
                t = xw.tile([128, 2, S], BF16, tag="xw",
                            name=f"ld_{param.name}_{qtr}")
                nc.gpsimd.dma_start(
                    out=t, in_=param[:, :].rearrange(
                        "(a p) s -> p a s", p=128)[:, 2 * qtr:2 * qtr + 2, :])
                return t

            for qtr in range(4):
                if qtr == 0:
                    nc.gpsimd.tensor_copy(junk_s, glast[7][0:1, 0, 0:2])
                elif qtr >= 2:
                    nc.gpsimd.tensor_copy(
                        junk_s, pieces[("wv", qtr - 2)][0:1, 0, 0:2])
                pieces[("xv", qtr)] = load_quarter_gp(xv, qtr)
                pieces[("wv", qtr)] = load_quarter_gp(wv, qtr)

            def mk_sl(param):
                ts4 = [pieces[(param.name, q)] for q in range(4)]
                def sl(k, cols, ts4=ts4):
                    return ts4[k // 2][:, k % 2, cols]
                return sl

            xv_s, wv_s = mk_sl(xv), mk_sl(wv)
            xq_s, wq_s = mk_sl(xq), mk_sl(wq)
            xk_s, wk_s = mk_sl(xk), mk_sl(wk)

            def va_slices(st, dc):
                dst = VA[:, st, :].rearrange("p (h w) -> p h w", w=65)
                return dst[:, dc * 8:(dc + 1) * 8, 0:64]

            def proj_qk(r, which, sc):
                w_s, x_s, dstT = ((wq_s, xq_s, QT) if which == 0
                                  else (wk_s, xk_s, KTt))
                pq = pp.tile([128, 512], F32, tag="pp")
                for k in range(KT):
                    nc.tensor.matmul(
                        pq[:, :],
                        lhsT=w_s(k, bass.ts(r, 128)),
                        rhs=x_s(k, bass.ts(sc, 512)),
                        start=(k == 0), stop=(k == KT - 1))
                nc.vector.tensor_copy(dstT[:, r, bass.ts(sc, 512)], pq)

            class ProjStepper:
                """Emit projection chains one matmul at a time so they pace
                evenly between attention chunks.

                Chain specs:
                  ('qk', which, sc, r)   -- 8 k-steps, full contraction
                  ('v', st, dc, half)    -- 4 k-steps (k = 4*half + j);
                     half 0 copies the partial sum into VA (bf16),
                     half 1 adds its partial sum on top (DVE tensor_add).
                """

                def __init__(self, chains, pool, tag, max_active=1):
                    self.pending = list(chains)
                    self.active = []   # [psum_tile, chain_spec, next_j]
                    self.rr = 0
                    self.pool, self.tag = pool, tag
                    self.MAX_ACTIVE = max_active

                def _start(self):
                    if self.pending:
                        spec = self.pending.pop(0)
                        pq = self.pool.tile([128, 512], F32, tag=self.tag,
                                            name=f"pq_{'_'.join(map(str, spec))}")
                        self.active.append([pq, spec, 0])

                def step(self, n=2):
                    for _ in range(n):
                        # keep TWO chains in flight and alternate their
                        # k-steps: consecutive matmuls accumulating into the
                        # SAME psum bank serialize fill/drain (~650ns/MM);
                        # alternating banks restores ~230ns/MM pipelining
                        while len(self.active) < self.MAX_ACTIVE and self.pending:
                            self._start()
                        if not self.active:
                            return
                        ent = self.active[self.rr % len(self.active)]
                        self.rr += 1
                        pq, spec, j = ent
                        if spec[0] == 'qk':
                            _, which, sc, r = spec
                            w_s, x_s = ((wq_s, xq_s) if which == 0
                                        else (wk_s, xk_s))
                            nsteps = KT
                            nc.tensor.matmul(
                                pq[:, :],
                                lhsT=w_s(j, bass.ts(r, 128)),
                                rhs=x_s(j, bass.ts(sc, 512)),
                                start=(j == 0), stop=(j == nsteps - 1))
                        else:
                            _, st, dc, half = spec
                            nsteps = 4
                            k = 4 * half + j
                            nc.tensor.matmul(
                                pq[:, :],
                                lhsT=xv_s(k, bass.ts(st, 128)),
                                rhs=wv_s(k, bass.ts(dc, 512)),
                                start=(j == 0), stop=(j == nsteps - 1))
                        ent[2] += 1
                        if ent[2] == nsteps:
                            if spec[0] == 'qk':
                                _, which, sc, r = spec
                                dstT = QT if which == 0 else KTt
                                nc.vector.tensor_copy(
                                    dstT[:, r, bass.ts(sc, 512)], pq)
                            else:
                                _, st, dc, half = spec
                                dst = va_slices(st, dc)
                                src = pq[:, :].rearrange(
                                    "p (h w) -> p h w", w=64)
                                if half == 0:
                                    nc.vector.tensor_copy(dst, src)
                                else:
                                    nc.vector.tensor_add(dst, dst, src)
                                    if dc == 1:
                                        ones = VA[:, st, :].rearrange(
                                            "p (h w) -> p h w", w=65)
                                        nc.vector.memset(
                                            ones[:, :, 64:65], 1.0)
                            self.active.remove(ent)

                def finish(self):
                    while self.active or self.pending:
                        self.step(1)

            # prelude: QT/KTt tile 0 ONLY — chain k-steps on the same PSUM
            # bank serialize (~650ns/MM); a 2-active stepper alternates two
            # banks so consecutive matmuls pipeline (~230ns).  Pair-1 chains
            # are paced inside pair 0.
            pre = ProjStepper([('qk', which, sc, 0)
                               for which in (0, 1) for sc in (0, 1)],
                              pp, "pp", max_active=2)
            pre.finish()

            OPs = {(0, 0): None, (0, 1): None, (1, 0): None, (1, 1): None}

            def scores_exp(r, qc, c):
                ps = psc.tile([128, 1024], F32, tag="psc")
                nc.tensor.matmul(
                    ps[:, 0:512],
                    lhsT=KTt[0:64, r, bass.ts(c, 128)],
                    rhs=QT[0:64, r, bass.ts(qc, 512)],
                    start=True, stop=True, tile_position=(0, 0))
                nc.tensor.matmul(
                    ps[:, 512:1024],
                    lhsT=KTt[64:128, r, bass.ts(c, 128)],
                    rhs=QT[64:128, r, bass.ts(qc, 512)],
                    start=True, stop=True, tile_position=(64, 0))
                pt = ptp.tile([128, 1024], BF16, tag="pt")
                nc.scalar.activation(pt, ps, AF.Exp,
                                     bias=mask_sb[:, c:c + 1], scale=1.0)
                return pt

            def attnv_chunk(r, po1, po2, pt, c):
                # attnV: pt-block stationary, [V_h | ones] moving. The 8
                # LDWEIGHTS pipeline into the PE background weight buffer,
                # so the whole burst issues in ~214ns.
                for ph, (po, hh) in enumerate(((po1, 2 * r), (po2, 2 * r + 1))):
                    for qt in range(4):
                        # start=True clears the WHOLE PSUM bank, so only the
                        # first slice-write of the group may set it; qt>0
                        # fresh-writes via per-element has_written instead.
                        nc.tensor.matmul(
                            po[:, qt, :],
                            lhsT=pt[:, ph * 512 + qt * 128:
                                    ph * 512 + (qt + 1) * 128],
                            rhs=VA[:, c, hh * 65:(hh + 1) * 65],
                            start=(c == 0 and qt == 0),
                            stop=(c == KT - 1))

            def outphase(r, qc, po1, po2):
                # denominator reciprocal + scale. Results for 4 consecutive
                # pairs are accumulated into one bf16 OP tile (512 output
                # columns -> 1KB DMA bursts; gpsimd DMA casts to fp32).
                rh = r // 4
                OP = OPs[(rh, qc)]
                if OP is None:
                    OP = OPs[(rh, qc)] = outp.tile(
                        [128, 4, 512], BF16, tag="outp", name=f"OP{rh}_{qc}")
                tail = (r == 7 and qc == 1)
                # The po psum banks are aliased by the NEXT group's attnV
                # (pod bufs=2 with 2 tiles/group = zero double buffering), so
                # drain them FAST: one bulk DVE copy each to SBUF (~0.25us),
                # then normalize out of SBUF on the idle gpsimd engine
                # (gpsimd cannot read PSUM directly).
                sb1 = rrp.tile([128, 4, 65], F32, tag="sbp", bufs=4,
                               name=f"sb1_{r}_{qc}")
                sb2 = rrp.tile([128, 4, 65], F32, tag="sbp", bufs=4,
                               name=f"sb2_{r}_{qc}")
                nc.vector.tensor_copy(sb1, po1)
                nc.vector.tensor_copy(sb2, po2)
                rr1 = rrp.tile([128, 4, 1], F32, tag="rr", bufs=4,
                               name=f"rr1_{r}_{qc}")
                rr2 = rrp.tile([128, 4, 1], F32, tag="rr", bufs=4,
                               name=f"rr2_{r}_{qc}")
                nc.vector.reciprocal(rr1, sb1[:, :, 64:65])
                nc.vector.reciprocal(rr2, sb2[:, :, 64:65])
                for qt in range(4):
                    for ph, (sb, rr) in enumerate(((sb1, rr1), (sb2, rr2))):
                        nc.gpsimd.tensor_scalar_mul(
                            OP[:, qt, (r % 4) * 128 + ph * DH:
                               (r % 4) * 128 + (ph + 1) * DH],
                            sb[:, qt, 0:64], rr[:, qt, 0:1])
                    if tail:
                        # last tile: DMA row-block by row-block as the
                        # divisions finish, to shorten the exposed tail
                        nc.gpsimd.dma_start(
                            out=out[qc * 512 + qt * 128:
                                    qc * 512 + (qt + 1) * 128,
                                    bass.ts(rh, 512)],
                            in_=OP[:, qt, :])
                if r % 4 == 3 and not tail:
                    nc.gpsimd.dma_start(
                        out=out[bass.ts(qc, 512), bass.ts(rh, 512)].rearrange(
                            "(a p) w -> p a w", p=128),
                        in_=OP[:, :, :])
                    OPs[(rh, qc)] = None

            # ---- main loop: uniform software pipeline over 16 qc-groups.
            # The exp stream (the ACT pacer) runs continuously; attnV for
            # group g-1 interleaves into group g's steps using pts held from
            # the previous group (peak 9 live pt tiles).  V chains pace into
            # groups 0 (A halves, k 0-3) and 1 (B halves, k 4-7, each
            # completing VA st=c right before attnv(group0, c) needs it).
            # Pair r+1's QK chains pace at 2/step through pair r's steps.
            stepA = ProjStepper([('v', st, dc, 0)
                                 for st in range(8) for dc in range(2)],
                                pod, "pod")
            stepB = ProjStepper([('v', st, dc, 1)
                                 for st in range(8) for dc in range(2)],
                                pp, "pp")
            qk_step = {}
            for rr_ in range(1, 8):
                qk_step[rr_] = ProjStepper([('qk', which, sc, rr_)
                                            for which in (0, 1)
                                            for sc in (0, 1)],
                                           pp, "pp")
            # attnV lags TWO groups (not one): group g's attnV runs during
            # group g+2, so the V projection gets 24 steps (groups 0-2) to
            # pace at 6-7 MM/step instead of cramming 8/step into g0/g1 at
            # ~3us/step while the exp pacer only needs 1.11us/step.  Only
            # one group's attnV is ever active, so pod residency and the
            # outphase pattern are unchanged; pts are held for two groups.
            heldq = []           # queue of (r, qc, pts), depth 2
            for g in range(16):
                r, qc = g // 2, g % 2
                cur = None
                if len(heldq) == 2:
                    hg = heldq.pop(0)
                    hp1 = pod.tile([128, 4, 65], F32, tag="pod",
                                   name=f"po1_g{g - 2}")
                    hp2 = pod.tile([128, 4, 65], F32, tag="pod",
                                   name=f"po2_g{g - 2}")
                    cur = (hg, hp1, hp2)
                pts = []
                for c in range(KT):
                    pts.append(scores_exp(r, qc, c))
                    if g == 0:
                        stepA.step(6)        # pod banks (attnV idle)
                        qk_step[1].step(2)   # pp banks
                    elif g == 1:
                        (stepA if c < 3 else stepB).step(7)
                        qk_step[1].step(2)
                    elif r < 7:
                        if g == 2:
                            stepB.step(6)    # VA st=c lands well before
                                             # attnv(g0, c) consumes it
                        # finish the next pair's chains by mid-group so the
                        # QT/KTt copies land well before its first scores
                        qk_step[r + 1].step(
                            2 if qc == 0 else
                            (3 if c < 4 else (2 if c < 6 else 0)))
                    if cur is not None:
                        attnv_chunk(cur[0][0], cur[1], cur[2],
                                    cur[0][2][c], c)
                if cur is not None:
                    outphase(cur[0][0], cur[0][1], cur[1], cur[2])
                heldq.append((r, qc, pts))
            # drain: last two groups' attnV + out-phase
            for i, hg in enumerate(heldq):
                hp1 = pod.tile([128, 4, 65], F32, tag="pod", name=f"po1_d{i}")
                hp2 = pod.tile([128, 4, 65], F32, tag="pod", name=f"po2_d{i}")
                for c in range(KT):
                    attnv_chunk(hg[0], hp1, hp2, hg[2][c], c)
                outphase(hg[0], hg[1], hp1, hp2)

    _split_excess_waits(nc)
    return nc


def _prep_inputs(queries, keys, values, valid_lens, w_q, w_k, w_v):
    bf = ml_dtypes.bfloat16
    wq_b = np.ascontiguousarray((w_q.astype(np.float32) / np.sqrt(DH)).astype(bf))
    wk_b = np.ascontiguousarray(w_k.astype(np.float32).astype(bf))
    wv_b = np.ascontiguousarray(w_v.astype(np.float32).astype(bf))
    idn = np.eye(128, dtype=bf)
    in_maps = []
    for b in range(B):
        mask = np.where(np.arange(S) < int(valid_lens[b]), 0.0, NEG)
        mask = np.ascontiguousarray(
            mask.reshape(KT, 128).T.astype(np.float32))          # [128, KT]
        in_maps.append(dict(
            xq=np.ascontiguousarray(queries[b].astype(np.float32).T.astype(bf)),
            xk=np.ascontiguousarray(keys[b].astype(np.float32).T.astype(bf)),
            xv=np.ascontiguousarray(values[b].astype(np.float32).T.astype(bf)),
            wq=wq_b, wk=wk_b, wv=wv_b, msk=mask, idn=idn,
        ))
    return in_maps


def kernel(queries, keys, values, valid_lens, w_q, w_k, w_v, _want_results=False):
    queries = np.asarray(queries)
    keys = np.asarray(keys)
    values = np.asarray(values)
    valid_lens = np.asarray(valid_lens)
    w_q, w_k, w_v = np.asarray(w_q), np.asarray(w_k), np.asarray(w_v)
    if "nc" not in _cache:
        _cache["nc"] = _build_program()
    nc = _cache["nc"]
    in_maps = _prep_inputs(queries, keys, values, valid_lens, w_q, w_k, w_v)
    res = run_bass_kernel_spmd(nc, in_maps, list(range(N_CORES)))
    out = np.stack([res.results[b]["out"] for b in range(B)]).astype(np.float32)
    # valid_len == 0: reference softmaxes an all -1e9 row -> uniform attention.
    for b in range(B):
        if int(valid_lens[b]) == 0:
            vfull = values[b].astype(np.float32) @ w_v.astype(np.float32)
            out[b] = np.broadcast_to(vfull.mean(axis=0), (S, D))
    if _want_results:
        return out, res
    return out

